# revision 14
# baseline (speedup 1.0000x reference)
"""Cross-head online Hadamard transform on 8 TRN2 NeuronCores.

Computes y = einsum('hk,bkd->bhd', had_K, x.reshape(-1, 32, 128)) / sqrt(32),
reshaped back to x's shape, for x of shape (4, 4096, 4096) fp32 and
had_K of shape (32, 32).

Strategy (data-parallel over tokens, bf16 I/O, host-marshalled layout):
  - Flatten x to (16384, 4096) tokens; shard 2048 tokens per core.
  - The op is memory-bound. The per-core DMA bus (~310-360 GB/s shared
    by both directions) sets the floor: 64 MB fp32 traffic/core would
    be ~205 us. The correctness tolerance (rel err < 2e-2) leaves ample
    room for bf16, which halves HBM bytes and makes the matmul
    full-rate (1 cycle/row vs 4 for fp32). x is cast to bf16 on the
    host; y is produced as bf16 on device and upcast on the host.
    Measured rel err ~2.4e-3.
  - The matmul needs heads on SBUF partitions, which in token-major
    DRAM layout forces 256-byte DMA descriptor chunks (~10% bandwidth
    penalty). Since the host already touches every element for the
    bf16 cast, it also pre-permutes each core's shard into the exact
    SBUF tile layout, so the device input DMA is fully contiguous
    (16 KB/partition runs); the device likewise writes its natural
    tile layout contiguously and the host un-permutes y. Each partition
    of each 512-token tile is a single 32 KB contiguous DMA run (one
    SDMA descriptor, just under the 64 KB cap). Measured ~96-103
    us/core across windows (chunked-descriptor version: ~112-117).
  - Per core, 4 SBUF tiles of 512 tokens, each [128, 2, 8192] bf16
    (32 KB/partition contiguous DMA runs), laid out [(j k), h, (g ti d)]
    with token t = t0 + h*256 + g*16 + ti*4 + j, head k, head-dim d.
    A single 128x128 stationary weight W = kron(I4, had_K.T)/sqrt(32)
    (bf16) mixes heads for 4 tokens at a time:
        out[(j h), (ti d)] = sum_{(j' k)} W[(j' k),(j h)] * in[(j' k),(ti d)]
    Per tile: 8 quarters x 4 matmuls (N=512) fill 4-bank fp32 PSUM
    tiles, copied to a bf16 SBUF tile (split ScalarE/VectorE) and
    DMA'd out.
  - Input DMA on the SP HWDGE queue, output DMA on the Activation
    HWDGE queue (measured fastest; gpsimd/SWDGE queues and
    finer-grained DMA splits are all slower).
"""

import math

import numpy as np
import ml_dtypes

N_CORES = 8
BATCH, SEQ, HIDDEN = 4, 4096, 4096
NUM_HEADS, HEAD_DIM = 32, 128
TOKENS = BATCH * SEQ                 # 16384
TOK_PER_CORE = TOKENS // N_CORES     # 2048
MACRO = 256                          # tokens per macro-tile
N_MACRO = TOK_PER_CORE // MACRO      # 8
ROWS = N_MACRO * 128                 # 1024 device DRAM rows per core
FREE = (MACRO // 16) * 4 * HEAD_DIM  # 8192 elems per row

_CACHE = {}


def _build(repeats=1):
    """Build the per-core Bass program. `repeats` re-runs the whole
    workload inside the NEFF (used only for benchmarking slope)."""
    import concourse.bacc as bacc
    import concourse.mybir as mybir
    from concourse import tile

    nc = bacc.Bacc("TRN2", target_bir_lowering=False, debug=False)
    f32 = mybir.dt.float32
    bf16 = mybir.dt.bfloat16

    # x/y are in pre-permuted tile layout: row m*128 + (j*32+k) holds the
    # (h g ti d) free elements of partition (j,k) of 512-token tile m —
    # one 32 KB contiguous DMA run per partition per tile.
    x = nc.dram_tensor("x", [ROWS // 2, FREE * 2], bf16, kind="ExternalInput")
    w = nc.dram_tensor("w", [128, 128], bf16, kind="ExternalInput")
    y = nc.dram_tensor("y", [ROWS // 2, FREE * 2], bf16, kind="ExternalOutput")

    xv = x.rearrange("(m p) (h f) -> m p h f", p=128, h=2)
    yq = y.rearrange("(m p) (h s f) -> m p h s f", p=128, h=2, s=2)

    with tile.TileContext(nc) as tc:
        with (
            tc.tile_pool(name="const", bufs=1) as pconst,
            tc.tile_pool(name="pin", bufs=3) as pin,
            tc.tile_pool(name="pout", bufs=3) as pout,
            tc.tile_pool(name="ppsum", bufs=2, space="PSUM") as ppsum,
        ):
            w_sb = pconst.tile([128, 128], bf16)
            nc.sync.dma_start(w_sb[:], w[:])

            for m in [m for _ in range(repeats) for m in range(N_MACRO // 2)]:
                in_t = pin.tile([128, 2, FREE], bf16)
                nc.sync.dma_start(in_t[:], xv[m])

                out_t = pout.tile([128, 2, 2, FREE // 2], bf16)
                qi = 0
                for h in range(2):
                    for s in range(4):
                        base = s * 2048
                        ps = ppsum.tile([128, 2048], f32)
                        for g in range(4):
                            nc.tensor.matmul(
                                ps[:, g * 512:(g + 1) * 512],
                                w_sb[:],
                                in_t[:, h, base + g * 512:base + (g + 1) * 512],
                                start=True,
                                stop=True,
                            )
                        v = out_t[:, h].rearrange("p a b -> p (a b)")
                        nc.scalar.copy(v[:, base:base + 1024], ps[:, :1024])
                        nc.vector.tensor_copy(v[:, base + 1024:base + 2048],
                                              ps[:, 1024:])
                        # fire each quarter-pair's 1 MB output as soon as
                        # its copies land, alternating queues so the
                        # write streams transfer concurrently
                        if s % 2 == 1:
                            eng = nc.scalar if qi % 2 == 0 else nc.sync
                            eng.dma_start(yq[m, :, h, s // 2],
                                          out_t[:, h, s // 2])
                            qi += 1

    nc.compile()
    return nc


def _get_nc(repeats=1):
    key = ("nc", repeats)
    if key not in _CACHE:
        _CACHE[key] = _build(repeats)
    return _CACHE[key]


def make_w(had_K):
    """Host-side stationary weight: kron(I4, had_K.T)/sqrt(32) in bf16."""
    scale = 1.0 / math.sqrt(NUM_HEADS)
    w_np = np.kron(np.eye(4, dtype=np.float32),
                   np.asarray(had_K, np.float32).T * scale)
    return np.ascontiguousarray(w_np).astype(ml_dtypes.bfloat16)


def prep_x(xs_bf16):
    """[2048, 4096] token-major -> [512, 16384] device tile layout.
    t = m*512 + h*256 + g*16 + ti*4 + j ; row = m*128 + j*32 + k ;
    col = (h g ti d)."""
    v = xs_bf16.reshape(N_MACRO // 2, 2, 16, 4, 4, NUM_HEADS,
                        HEAD_DIM)                    # m h g ti j k d
    v = v.transpose(0, 4, 5, 1, 2, 3, 6)             # m j k h g ti d
    return np.ascontiguousarray(v).reshape(ROWS // 2, FREE * 2)


def unprep_y(yd_bf16):
    """[512, 16384] device tile layout -> [2048, 4096] token-major."""
    v = yd_bf16.reshape(N_MACRO // 2, 4, NUM_HEADS, 2, 16, 4,
                        HEAD_DIM)                    # m j hh h g ti d
    v = v.transpose(0, 3, 4, 5, 1, 2, 6)             # m h g ti j hh d
    return np.ascontiguousarray(v).reshape(TOK_PER_CORE, HIDDEN)


def kernel(x, had_K):
    from concourse.bass_utils import run_bass_kernel_spmd

    x = np.asarray(x)
    init_shape = x.shape

    w_np = make_w(had_K)
    xt = np.ascontiguousarray(x.reshape(TOKENS, HIDDEN)).astype(
        ml_dtypes.bfloat16)
    in_maps = [
        {
            "x": prep_x(xt[i * TOK_PER_CORE:(i + 1) * TOK_PER_CORE]),
            "w": w_np,
        }
        for i in range(N_CORES)
    ]

    nc = _get_nc()
    res = run_bass_kernel_spmd(nc, in_maps, core_ids=list(range(N_CORES)))
    out = np.concatenate([unprep_y(res.results[i]["y"])
                          for i in range(N_CORES)], axis=0)
    return out.astype(np.float32).reshape(init_shape)
